# revision 15
# baseline (speedup 1.0000x reference)
"""Trainium2 Bass kernel for nn_Block_21028159881813 (dense transformer block).

Strategy: data-parallel over batch n=16 across 8 NeuronCores (2 elems/core).
Per element, three slab-pipelined passes (all matmuls bf16, fp32 PSUM):
  A: LN1 + PE-transpose + K/V projection + linear-attn context accumulation
  B: Q projection + softmax + attention + reprojection
  C: residual + LN2 + PE-transpose + fc1/gelu + fc2 + residual
LN work overlaps matmul work inside each pass (no standalone LN phases).

Key algebraic simplifications vs the reference:
  - keys bias bk cancels in softmax-over-L: skipped entirely.
  - values bias bv folded into normalized context: ctxn = ctx_raw/s + bv.
  - the reference's raw reshapes [L,D]<->[D,L] are free: both are contiguous
    views of the same flat buffer, handled by DMA access patterns.
  - 1/x computed as exp(-ln(x)) on ACT (DVE reciprocal breaks this walrus).
"""

import sys
import numpy as np

for _p in ("/opt/trn_rl_repo", "/opt/pypackages"):
    if _p not in sys.path:
        sys.path.insert(0, _p)

import ml_dtypes
import concourse.bass as bass
import concourse.mybir as mybir
import concourse.tile as tile
from concourse.bass_utils import run_bass_kernel_spmd

F32 = mybir.dt.float32
BF16 = mybir.dt.bfloat16
BF = ml_dtypes.bfloat16
Alu = mybir.AluOpType
Act = mybir.ActivationFunctionType

N, L, D, H = 16, 3136, 768, 8
K, V, M = 768, 384, 3072
hk, hv = K // H, V // H  # 96, 48
EPS = 1e-6
NB = 2          # batch elems per core
NCORES = 8

# (chunk c, head h, jmin, jmax, dst_p): v-cols 48h+j of head h that land in
# partition dst_p.. of v-chunk c (128 wide).
INCID = [
    (0, 0, 0, 48, 0), (0, 1, 0, 48, 48), (0, 2, 0, 32, 96),
    (1, 2, 32, 48, 0), (1, 3, 0, 48, 16), (1, 4, 0, 48, 64), (1, 5, 0, 16, 112),
    (2, 5, 16, 48, 0), (2, 6, 0, 48, 32), (2, 7, 0, 48, 80),
]


def _ltiles512():
    for it in range((L + 511) // 512):
        l0 = it * 512
        yield it, l0, min(512, L - l0)


def _layernorm_tile(nc, tp, xin, p, eps_t, g_b, b_b, out_t):
    """LN over free dim D=768 of xin[:p] (f32, NOT mutated) -> out_t[:p] bf16."""
    stats = tp.tile([128, 3, 6], F32, name="ln_stats")
    xg = xin[:p].rearrange("p (s c) -> p s c", c=256)
    for s in range(3):
        nc.vector.bn_stats(out=stats[:p, s], in_=xg[:, s])
    mv = tp.tile([128, 2], F32, name="ln_mv")
    nc.vector.bn_aggr(out=mv[:p], in_=stats[:p])
    # mv[:,1] = 1/sqrt(var + eps) = exp(-0.5 * ln(var + eps))
    nc.scalar.activation(out=mv[:p, 1:2], in_=mv[:p, 1:2], func=Act.Ln,
                         bias=eps_t[:p], scale=1.0)
    nc.scalar.activation(out=mv[:p, 1:2], in_=mv[:p, 1:2], func=Act.Exp,
                         scale=-0.5)
    # nmr = -mean * rstd; then normalize on ACT: y = x*rstd + nmr
    nc.vector.tensor_scalar(out=mv[:p, 0:1], in0=mv[:p, 0:1],
                            scalar1=mv[:p, 1:2], scalar2=-1.0,
                            op0=Alu.mult, op1=Alu.mult)
    nc.scalar.activation(out=out_t[:p], in_=xin[:p], func=Act.Identity,
                         bias=mv[:p, 0:1], scale=mv[:p, 1:2])
    nc.vector.tensor_mul(out=out_t[:p], in0=out_t[:p], in1=g_b[:p])
    nc.vector.tensor_add(out=out_t[:p], in0=out_t[:p], in1=b_b[:p])


def _build():
    nc = bass.Bass()

    x_in = nc.dram_tensor("x", [NB, L, D], F32, kind="ExternalInput")
    wkt = nc.dram_tensor("wkt", [D, K], BF16, kind="ExternalInput")
    wqt = nc.dram_tensor("wqt", [D, K], BF16, kind="ExternalInput")
    wvt = nc.dram_tensor("wvt", [D, V], BF16, kind="ExternalInput")
    wrt = nc.dram_tensor("wrt", [V, D], BF16, kind="ExternalInput")
    w1t = nc.dram_tensor("w1t", [D, M], BF16, kind="ExternalInput")
    w2t = nc.dram_tensor("w2t", [M, D], BF16, kind="ExternalInput")
    bq96 = nc.dram_tensor("bq96", [hk, H], F32, kind="ExternalInput")
    bv848 = nc.dram_tensor("bv848", [H, hv], F32, kind="ExternalInput")
    br6 = nc.dram_tensor("br6", [128, 6], F32, kind="ExternalInput")
    b1c = nc.dram_tensor("b1c", [128, 24], F32, kind="ExternalInput")
    b2v = nc.dram_tensor("b2v", [D], F32, kind="ExternalInput")
    ln1g = nc.dram_tensor("ln1g", [D], BF16, kind="ExternalInput")
    ln1b = nc.dram_tensor("ln1b", [D], BF16, kind="ExternalInput")
    ln2g = nc.dram_tensor("ln2g", [D], BF16, kind="ExternalInput")
    ln2b = nc.dram_tensor("ln2b", [D], BF16, kind="ExternalInput")
    mskd = nc.dram_tensor("msk", [hk, len(INCID), 128], BF16, kind="ExternalInput")
    identd = nc.dram_tensor("ident", [128, 128], BF16, kind="ExternalInput")
    out_d = nc.dram_tensor("out", [NB, L, D], F32, kind="ExternalOutput")

    def bcast(src, P, n):
        return bass.AP(tensor=src.tensor, offset=src.offset, ap=[[0, P], [1, n]])

    with tile.TileContext(nc) as tc:
        from contextlib import ExitStack
        with ExitStack() as top:
            wp = top.enter_context(tc.tile_pool(name="wts", bufs=1))
            dp = top.enter_context(tc.tile_pool(name="dram", bufs=2, space="DRAM"))

            # ---- resident weights (bf16), chunked [128, nchunks, cols]
            wk_sb = wp.tile([128, 6, K], BF16)
            nc.sync.dma_start(out=wk_sb, in_=wkt.rearrange("(c p) k -> p c k", p=128))
            wq_sb = wp.tile([128, 6, K], BF16)
            nc.sync.dma_start(out=wq_sb, in_=wqt.rearrange("(c p) k -> p c k", p=128))
            wv_sb = wp.tile([128, 6, V], BF16)
            nc.sync.dma_start(out=wv_sb, in_=wvt.rearrange("(c p) k -> p c k", p=128))
            wr_sb = wp.tile([128, 3, D], BF16)
            nc.sync.dma_start(out=wr_sb, in_=wrt.rearrange("(c p) k -> p c k", p=128))
            w1_sb = wp.tile([128, 6, M], BF16)
            nc.sync.dma_start(out=w1_sb, in_=w1t.rearrange("(c p) k -> p c k", p=128))
            w2_sb = wp.tile([128, 24, D], BF16)
            nc.sync.dma_start(out=w2_sb, in_=w2t.rearrange("(c p) k -> p c k", p=128))

            # ---- resident small constants
            bq_sb = wp.tile([hk, H], F32)
            nc.sync.dma_start(out=bq_sb, in_=bq96[:, :])
            bvb = wp.tile([hk, H, hv], F32)
            _bv = bv848[:, :]
            nc.sync.dma_start(out=bvb, in_=bass.AP(
                tensor=_bv.tensor, offset=_bv.offset, ap=[[0, hk], [hv, H], [1, hv]]))
            br_sb = wp.tile([128, 6], F32)
            nc.sync.dma_start(out=br_sb, in_=br6[:, :])
            b1_sb = wp.tile([128, 24], F32)
            nc.sync.dma_start(out=b1_sb, in_=b1c[:, :])
            b2b = wp.tile([128, D], F32)
            nc.sync.dma_start(out=b2b, in_=bcast(b2v[:], 128, D))
            g1b = wp.tile([128, D], BF16)
            nc.sync.dma_start(out=g1b, in_=bcast(ln1g[:], 128, D))
            b1lb = wp.tile([128, D], BF16)
            nc.sync.dma_start(out=b1lb, in_=bcast(ln1b[:], 128, D))
            g2b = wp.tile([128, D], BF16)
            nc.sync.dma_start(out=g2b, in_=bcast(ln2g[:], 128, D))
            b2lb = wp.tile([128, D], BF16)
            nc.sync.dma_start(out=b2lb, in_=bcast(ln2b[:], 128, D))
            msk_sb = wp.tile([hk, len(INCID), 128], BF16)
            nc.sync.dma_start(out=msk_sb, in_=mskd[:, :, :])
            ident = wp.tile([128, 128], BF16)
            nc.sync.dma_start(out=ident, in_=identd[:, :])
            eps_t = wp.tile([128, 1], F32)
            nc.vector.memset(eps_t, EPS)

            lnp = top.enter_context(tc.tile_pool(name="lnp", bufs=2))
            W = dict(
                wk=wk_sb, wq=wq_sb, wv=wv_sb, wr=wr_sb, w1=w1_sb, w2=w2_sb,
                bq=bq_sb, bvb=bvb, br=br_sb, b1=b1_sb, b2b=b2b,
                g1b=g1b, b1lb=b1lb, g2b=g2b, b2lb=b2lb,
                msk=msk_sb, ident=ident, eps=eps_t, lnp=lnp)
            scrs = []
            for e in range(NB):
                scrs.append({
                    "y": dp.tile([D * L], BF16, name="y_scr"),
                    "attn": dp.tile([D * L], BF16, name="attn_scr"),
                })
            # Pipelined emission: elem e+1's LN1 overlaps elem e's attention,
            # so the PE never waits on a standalone LN phase after startup.
            _emit_elem_ln1(nc, tc, 0, x_in[0], scrs[0], W)
            _emit_elem_attn(nc, tc, 0, x_in[0], out_d[0], scrs[0], W)
            _emit_elem_ln1(nc, tc, 1, x_in[1], scrs[1], W)
            _emit_elem_mlp(nc, tc, 0, x_in[0], out_d[0], scrs[0], W)
            _emit_elem_attn(nc, tc, 1, x_in[1], out_d[1], scrs[1], W)
            _emit_elem_mlp(nc, tc, 1, x_in[1], out_d[1], scrs[1], W)
    return nc


def _emit_elem_ln1(nc, tc, e, x_e, scr, W):
    """LN1: x -> y (bf16, [L, D] rows). Emitted early so elem e's LN overlaps
    the previous elem's attention/MLP passes (shared top-level pool)."""
    y_ld = scr["y"].rearrange("(l d) -> l d", d=D)
    lp = W["lnp"]
    for it in range((L + 127) // 128):
        l0 = it * 128
        p = min(128, L - l0)
        xt = lp.tile([128, D], F32, name="xt1")
        nc.sync.dma_start(out=xt[:p], in_=x_e[l0:l0 + p, :])
        y1 = lp.tile([128, D], BF16, name="y1")
        _layernorm_tile(nc, lp, xt, p, W["eps"], W["g1b"], W["b1lb"], y1)
        nc.sync.dma_start(out=y_ld[l0:l0 + p, :], in_=y1[:p])


def _emit_elem_attn(nc, tc, e, x_e, out_e, scr, W):
    from contextlib import ExitStack

    y_dl6 = scr["y"].rearrange("(c p l) -> p c l", p=128, l=L)
    attn_dl = scr["attn"].rearrange("(d l) -> d l", l=L)

    # small pool spanning passes A and B (ctxn/cpd)
    phAB = ExitStack()
    cp = phAB.enter_context(tc.tile_pool(name=f"pABc_{e}", bufs=1))

    # ============ pass A: K/V projection + linear-attn context ============
    with ExitStack() as phA:
        zp = phA.enter_context(tc.tile_pool(name=f"pAz_{e}", bufs=2))
        ep = phA.enter_context(tc.tile_pool(name=f"pAe_{e}", bufs=3))
        vp = phA.enter_context(tc.tile_pool(name=f"pAv_{e}", bufs=3))
        sp = phA.enter_context(tc.tile_pool(name=f"pAs_{e}", bufs=1))
        kp = phA.enter_context(tc.tile_pool(name=f"pAkp_{e}", bufs=2, space="PSUM"))
        vpp = phA.enter_context(tc.tile_pool(name=f"pAvp_{e}", bufs=2, space="PSUM"))
        cxp = phA.enter_context(tc.tile_pool(name=f"pAcx_{e}", bufs=1, space="PSUM"))

        ctx_ps = cxp.tile([hk, H, hv + 1], F32)
        ctx_flat = ctx_ps.rearrange("p a b -> p (a b)")
        zero96 = sp.tile([hk, hk], BF16)
        nc.vector.memset(zero96, 0.0)
        junk = sp.tile([hk, H * (hv + 1)], BF16)
        nc.vector.memset(junk, 0.0)
        # open the psum accumulation region with an all-zero write
        nc.tensor.matmul(out=ctx_flat, lhsT=zero96, rhs=junk, start=True, stop=False)

        for it5, l0, lw in _ltiles512():
            zsl = zp.tile([128, 6, 512], BF16, name="zsl")
            nc.sync.dma_start(out=zsl[:, :, :lw], in_=y_dl6[:, :, l0:l0 + lw])
            for sb in range((lw + 127) // 128):
                p = min(128, lw - sb * 128)
                lo = sb * 128
                kps = kp.tile([128, K], F32, name="kps")
                vps = vpp.tile([128, V], F32, name="vps")
                for c0, c1 in ((0, 512), (512, 768)):
                    for dc in range(6):
                        nc.tensor.matmul(out=kps[:p, c0:c1],
                                         lhsT=zsl[:, dc, lo:lo + p],
                                         rhs=W["wk"][:, dc, c0:c1],
                                         start=(dc == 0), stop=(dc == 5))
                for dc in range(6):
                    nc.tensor.matmul(out=vps[:p], lhsT=zsl[:, dc, lo:lo + p],
                                     rhs=W["wv"][:, dc, :],
                                     start=(dc == 0), stop=(dc == 5))
                ekt = ep.tile([128, K], BF16, name="ekt")
                nc.scalar.activation(out=ekt[:p], in_=kps[:p], func=Act.Exp)
                vt = vp.tile([128, H, hv + 1], BF16, name="vt")
                nc.vector.tensor_copy(
                    out=vt[:p, :, 0:hv],
                    in_=vps[:p].rearrange("p (a b) -> p a b", b=hv))
                nc.vector.memset(vt[:p, :, hv:hv + 1], 1.0)
                for h in range(H):
                    nc.tensor.matmul(out=ctx_ps[:, h, :],
                                     lhsT=ekt[:p, hk * h:hk * (h + 1)],
                                     rhs=vt[:p, h, :], start=False, stop=False)
        # close the accumulation region (+0)
        nc.tensor.matmul(out=ctx_flat, lhsT=zero96, rhs=junk, start=False, stop=True)

        # finalize: ctxn = ctx_raw / s + bv
        ctxs = sp.tile([hk, H, hv + 1], F32)
        nc.vector.tensor_copy(out=ctxs, in_=ctx_ps)
        ctxn = cp.tile([hk, H, hv], BF16)
        for h in range(H):
            rec = sp.tile([hk, 1], F32, name=f"rec{h}")
            nc.scalar.activation(out=rec, in_=ctxs[:, h, hv:hv + 1], func=Act.Ln)
            nc.scalar.activation(out=rec, in_=rec, func=Act.Exp, scale=-1.0)
            nc.vector.scalar_tensor_tensor(
                out=ctxn[:, h, :], in0=ctxs[:, h, 0:hv], scalar=rec,
                in1=W["bvb"][:, h, :], op0=Alu.mult, op1=Alu.add)

    # ============ pass B: Q proj + attention + reprojection ============
    with phAB as phB:
        cpd = cp.tile([hk, len(INCID), 128], BF16)
        nc.vector.memset(cpd, 0.0)
        for i, (c, h, jmin, jmax, dstp) in enumerate(INCID):
            nc.vector.tensor_copy(out=cpd[:, i, dstp:dstp + (jmax - jmin)],
                                  in_=ctxn[:, h, jmin:jmax])

        zp = phB.enter_context(tc.tile_pool(name=f"pBz_{e}", bufs=2))
        eqp = phB.enter_context(tc.tile_pool(name=f"pBe_{e}", bufs=2))
        rp = phB.enter_context(tc.tile_pool(name=f"pBr_{e}", bufs=2))
        ap_ = phB.enter_context(tc.tile_pool(name=f"pBa_{e}", bufs=2))
        rot = phB.enter_context(tc.tile_pool(name=f"pBo_{e}", bufs=2))
        qp = phB.enter_context(tc.tile_pool(name=f"pBqp_{e}", bufs=2, space="PSUM"))
        sqp = phB.enter_context(tc.tile_pool(name=f"pBsp_{e}", bufs=2, space="PSUM"))
        atp = phB.enter_context(tc.tile_pool(name=f"pBap_{e}", bufs=2, space="PSUM"))
        rop = phB.enter_context(tc.tile_pool(name=f"pBrp_{e}", bufs=2, space="PSUM"))

        for it5, l0, lw in _ltiles512():
            zt = zp.tile([128, 6, 512], BF16, name="zt")
            nc.sync.dma_start(out=zt[:, :, :lw], in_=y_dl6[:, :, l0:l0 + lw])
            eq = eqp.tile([hk, H, 512], BF16, name="eq")
            for h in range(H):
                qps = qp.tile([hk, 512], F32, name="qps")
                for dc in range(6):
                    nc.tensor.matmul(out=qps[:, :lw],
                                     lhsT=W["wq"][:, dc, hk * h:hk * (h + 1)],
                                     rhs=zt[:, dc, :lw],
                                     start=(dc == 0), stop=(dc == 5))
                nc.scalar.activation(out=eq[:, h, :lw], in_=qps[:, :lw],
                                     func=Act.Exp, bias=W["bq"][:, h:h + 1],
                                     scale=1.0)
            rqb = rp.tile([128, 3, 512], F32, name="rqb")
            attn_sb = ap_.tile([128, 3, 512], BF16, name="attn_sb")
            for c in range(3):
                inc = [i for i, t in enumerate(INCID) if t[0] == c]
                sqps = sqp.tile([128, 512], F32, name="sqps")
                for j, i in enumerate(inc):
                    h = INCID[i][1]
                    nc.tensor.matmul(out=sqps[:, :lw], lhsT=W["msk"][:, i, :],
                                     rhs=eq[:, h, :lw],
                                     start=(j == 0), stop=(j == len(inc) - 1))
                nc.scalar.activation(out=rqb[:, c, :lw], in_=sqps[:, :lw],
                                     func=Act.Ln)
                nc.scalar.activation(out=rqb[:, c, :lw], in_=rqb[:, c, :lw],
                                     func=Act.Exp, scale=-1.0)
                atps = atp.tile([128, 512], F32, name="atps")
                for j, i in enumerate(inc):
                    h = INCID[i][1]
                    nc.tensor.matmul(out=atps[:, :lw], lhsT=cpd[:, i, :],
                                     rhs=eq[:, h, :lw],
                                     start=(j == 0), stop=(j == len(inc) - 1))
                nc.vector.tensor_mul(out=attn_sb[:, c, :lw], in0=atps[:, :lw],
                                     in1=rqb[:, c, :lw])
            for dc in range(6):
                rops = rop.tile([128, 512], F32, name="rops")
                for c in range(3):
                    nc.tensor.matmul(out=rops[:, :lw],
                                     lhsT=W["wr"][:, c, dc * 128:(dc + 1) * 128],
                                     rhs=attn_sb[:, c, :lw],
                                     start=(c == 0), stop=(c == 2))
                ro = rot.tile([128, 512], BF16, name="ro")
                nc.vector.tensor_scalar_add(out=ro[:, :lw], in0=rops[:, :lw],
                                            scalar1=W["br"][:, dc:dc + 1])
                nc.sync.dma_start(out=attn_dl[dc * 128:(dc + 1) * 128, l0:l0 + lw],
                                  in_=ro[:, :lw])


def _emit_elem_mlp(nc, tc, e, x_e, out_e, scr, W):
    """Pass C: residual + LN2 + transpose + fc1/gelu + fc2 + residual."""
    from contextlib import ExitStack
    attn_ld = scr["attn"].rearrange("(l d) -> l d", d=D)
    with ExitStack() as phC:
        lp = phC.enter_context(tc.tile_pool(name=f"pCl_{e}", bufs=3))
        x2p = phC.enter_context(tc.tile_pool(name=f"pCx_{e}", bufs=2))
        y2p = phC.enter_context(tc.tile_pool(name=f"pCy_{e}", bufs=1))
        gp = phC.enter_context(tc.tile_pool(name=f"pCg_{e}", bufs=1))
        op = phC.enter_context(tc.tile_pool(name=f"pCo_{e}", bufs=2))
        tpp = phC.enter_context(tc.tile_pool(name=f"pCtp_{e}", bufs=1, space="PSUM"))
        f1p = phC.enter_context(tc.tile_pool(name=f"pCf1_{e}", bufs=3, space="PSUM"))
        f2p = phC.enter_context(tc.tile_pool(name=f"pCf2_{e}", bufs=2, space="PSUM"))

        for it5, l0, lw in _ltiles512():
            nsub = (lw + 127) // 128
            x2sl = x2p.tile([128, 4, D], F32, name="x2sl")
            y2sl = y2p.tile([128, 6, 512], BF16, name="y2sl")
            for sb in range(nsub):
                p = min(128, lw - sb * 128)
                lo = sb * 128
                gl0 = l0 + lo
                at = lp.tile([128, D], BF16, name="at")
                nc.sync.dma_start(out=at[:p], in_=attn_ld[gl0:gl0 + p, :])
                nc.sync.dma_start(out=x2sl[:p, sb, :], in_=x_e[gl0:gl0 + p, :])
                # x2 = attn + x  (f32 += bf16)
                nc.vector.tensor_add(out=x2sl[:p, sb, :], in0=x2sl[:p, sb, :],
                                     in1=at[:p])
                y2 = lp.tile([128, D], BF16, name="y2")
                _layernorm_tile(nc, lp, x2sl[:, sb, :], p, W["eps"],
                                W["g2b"], W["b2lb"], y2)
                tps = tpp.tile([128, 6, 128], BF16, name="tpsC")
                for dc in range(6):
                    nc.tensor.transpose(out=tps[:, dc, :p],
                                        in_=y2[:p, dc * 128:(dc + 1) * 128],
                                        identity=W["ident"][:p, :p])
                nc.vector.tensor_copy(out=y2sl[:, :, lo:lo + p], in_=tps[:, :, :p])
            G = gp.tile([128, 24, 512], BF16, name="G")
            for mc in range(24):
                f1 = f1p.tile([128, 512], F32, name="f1")
                for dc in range(6):
                    nc.tensor.matmul(out=f1[:, :lw],
                                     lhsT=W["w1"][:, dc, mc * 128:(mc + 1) * 128],
                                     rhs=y2sl[:, dc, :lw],
                                     start=(dc == 0), stop=(dc == 5))
                nc.scalar.activation(out=G[:, mc, :lw], in_=f1[:, :lw],
                                     func=Act.Gelu, bias=W["b1"][:, mc:mc + 1],
                                     scale=1.0)
            for sb in range(nsub):
                p = min(128, lw - sb * 128)
                lo = sb * 128
                gl0 = l0 + lo
                f2 = f2p.tile([128, D], F32, name="f2")
                for c0, c1 in ((0, 512), (512, 768)):
                    for mc in range(24):
                        nc.tensor.matmul(out=f2[:p, c0:c1],
                                         lhsT=G[:, mc, lo:lo + p],
                                         rhs=W["w2"][:, mc, c0:c1],
                                         start=(mc == 0), stop=(mc == 23))
                ot = op.tile([128, D], F32, name="ot")
                nc.vector.tensor_add(out=ot[:p], in0=f2[:p], in1=W["b2b"][:p])
                nc.vector.tensor_add(out=ot[:p], in0=ot[:p], in1=x2sl[:p, sb, :])
                nc.sync.dma_start(out=out_e[gl0:gl0 + p, :], in_=ot[:p])


def _legalize_single_wait(nc):
    """This walrus build encodes at most ONE sync wait per instruction
    (raw-bass style: waits are standalone InstEventSemaphore). Tile attaches
    multi-waits directly to instructions; hoist the extras onto EventSemaphore
    instructions inserted just before, on the same engine stream."""
    n = 0
    for f in nc.m.functions:
        for b in f.blocks:
            out = []
            changed = False
            for inst in b.instructions:
                si = inst.sync_info
                waits = list(si.on_wait) if si is not None and si.on_wait else []
                if len(waits) > 1:
                    changed = True
                    for w in waits[:-1]:
                        n += 1
                        ev = mybir.InstEventSemaphore(
                            name=f"EVLEG-{n}", ins=[], outs=[])
                        ev.engine = inst.engine
                        ev.sync_info = mybir.SyncInfo(on_wait=[w], on_update=[])
                        out.append(ev)
                    try:
                        si.on_wait = [waits[-1]]
                    except Exception:
                        inst.sync_info = mybir.SyncInfo(
                            on_wait=[waits[-1]],
                            on_update=list(si.on_update) if si.on_update else [])
                out.append(inst)
            if changed:
                b.instructions = out
    return n


_PROGRAM = None


def _get_program():
    global _PROGRAM
    if _PROGRAM is None:
        _PROGRAM = _build()
        _legalize_single_wait(_PROGRAM)
    return _PROGRAM


def _prep_common(inputs):
    f32 = np.float32
    g = lambda k: np.asarray(inputs[k], dtype=f32)
    msk = np.zeros((hk, len(INCID), 128), dtype=BF)
    for i, (c, h, jmin, jmax, dstp) in enumerate(INCID):
        msk[:, i, dstp:dstp + (jmax - jmin)] = 1
    return {
        "wkt": np.ascontiguousarray(g("Wk").T).astype(BF),
        "wqt": np.ascontiguousarray(g("Wq").T).astype(BF),
        "wvt": np.ascontiguousarray(g("Wv").T).astype(BF),
        "wrt": np.ascontiguousarray(g("Wr").T).astype(BF),
        "w1t": np.ascontiguousarray(g("W1").T).astype(BF),
        "w2t": np.ascontiguousarray(g("W2").T).astype(BF),
        "bq96": np.ascontiguousarray(g("bq").reshape(H, hk).T),
        "bv848": np.ascontiguousarray(g("bv").reshape(H, hv)),
        "br6": np.ascontiguousarray(g("br").reshape(6, 128).T),
        "b1c": np.ascontiguousarray(g("b1").reshape(24, 128).T),
        "b2v": g("b2"),
        "ln1g": g("ln1_g").astype(BF), "ln1b": g("ln1_b").astype(BF),
        "ln2g": g("ln2_g").astype(BF), "ln2b": g("ln2_b").astype(BF),
        "msk": msk,
        "ident": np.eye(128, dtype=BF),
    }


def kernel(**inputs):
    nc = _get_program()
    common = _prep_common(inputs)
    x = np.asarray(inputs["x"], dtype=np.float32)
    in_maps = [dict(common, x=np.ascontiguousarray(x[NB * i:NB * (i + 1)]))
               for i in range(NCORES)]
    res = run_bass_kernel_spmd(nc, in_maps, list(range(NCORES)))
    out = np.concatenate([res.results[i]["out"] for i in range(NCORES)], axis=0)
    return out.astype(np.float32)


if __name__ == "__main__":
    nc = _build()
    n = _legalize_single_wait(nc)
    print("built ok; hoisted waits:", n)


# revision 16
# speedup vs baseline: 1.0347x; 1.0347x over previous
"""Trainium2 Bass kernel for nn_Block_21028159881813 (dense transformer block).

Strategy: data-parallel over batch n=16 across 8 NeuronCores (2 elems/core).
Per element, three slab-pipelined passes (all matmuls bf16, fp32 PSUM):
  A: LN1 + PE-transpose + K/V projection + linear-attn context accumulation
  B: Q projection + softmax + attention + reprojection
  C: residual + LN2 + PE-transpose + fc1/gelu + fc2 + residual
LN work overlaps matmul work inside each pass (no standalone LN phases).

Key algebraic simplifications vs the reference:
  - keys bias bk cancels in softmax-over-L: skipped entirely.
  - values bias bv folded into normalized context: ctxn = ctx_raw/s + bv.
  - the reference's raw reshapes [L,D]<->[D,L] are free: both are contiguous
    views of the same flat buffer, handled by DMA access patterns.
  - 1/x computed as exp(-ln(x)) on ACT (DVE reciprocal breaks this walrus).
"""

import sys
import numpy as np

for _p in ("/opt/trn_rl_repo", "/opt/pypackages"):
    if _p not in sys.path:
        sys.path.insert(0, _p)

import ml_dtypes
import concourse.bass as bass
import concourse.mybir as mybir
import concourse.tile as tile
from concourse.bass_utils import run_bass_kernel_spmd

F32 = mybir.dt.float32
BF16 = mybir.dt.bfloat16
BF = ml_dtypes.bfloat16
Alu = mybir.AluOpType
Act = mybir.ActivationFunctionType

N, L, D, H = 16, 3136, 768, 8
K, V, M = 768, 384, 3072
hk, hv = K // H, V // H  # 96, 48
EPS = 1e-6
NB = 2          # batch elems per core
NCORES = 8

# (chunk c, head h, jmin, jmax, dst_p): v-cols 48h+j of head h that land in
# partition dst_p.. of v-chunk c (128 wide).
INCID = [
    (0, 0, 0, 48, 0), (0, 1, 0, 48, 48), (0, 2, 0, 32, 96),
    (1, 2, 32, 48, 0), (1, 3, 0, 48, 16), (1, 4, 0, 48, 64), (1, 5, 0, 16, 112),
    (2, 5, 16, 48, 0), (2, 6, 0, 48, 32), (2, 7, 0, 48, 80),
]


def _ltiles512():
    for it in range((L + 511) // 512):
        l0 = it * 512
        yield it, l0, min(512, L - l0)


def _layernorm_tile(nc, tp, xin, p, eps_t, g_b, b_b, out_t):
    """LN over free dim D=768 of xin[:p] (f32, NOT mutated) -> out_t[:p] bf16."""
    stats = tp.tile([128, 3, 6], F32, name="ln_stats")
    xg = xin[:p].rearrange("p (s c) -> p s c", c=256)
    for s in range(3):
        nc.vector.bn_stats(out=stats[:p, s], in_=xg[:, s])
    mv = tp.tile([128, 2], F32, name="ln_mv")
    nc.vector.bn_aggr(out=mv[:p], in_=stats[:p])
    # mv[:,1] = 1/sqrt(var + eps) = exp(-0.5 * ln(var + eps))
    nc.scalar.activation(out=mv[:p, 1:2], in_=mv[:p, 1:2], func=Act.Ln,
                         bias=eps_t[:p], scale=1.0)
    nc.scalar.activation(out=mv[:p, 1:2], in_=mv[:p, 1:2], func=Act.Exp,
                         scale=-0.5)
    # nmr = -mean * rstd; then normalize on ACT: y = x*rstd + nmr
    nc.vector.tensor_scalar(out=mv[:p, 0:1], in0=mv[:p, 0:1],
                            scalar1=mv[:p, 1:2], scalar2=-1.0,
                            op0=Alu.mult, op1=Alu.mult)
    nc.scalar.activation(out=out_t[:p], in_=xin[:p], func=Act.Identity,
                         bias=mv[:p, 0:1], scale=mv[:p, 1:2])
    nc.vector.tensor_mul(out=out_t[:p], in0=out_t[:p], in1=g_b[:p])
    nc.vector.tensor_add(out=out_t[:p], in0=out_t[:p], in1=b_b[:p])


def _build():
    nc = bass.Bass()

    x_in = nc.dram_tensor("x", [NB, L, D], F32, kind="ExternalInput")
    wkt = nc.dram_tensor("wkt", [D, K], BF16, kind="ExternalInput")
    wqt = nc.dram_tensor("wqt", [D, K], BF16, kind="ExternalInput")
    wvt = nc.dram_tensor("wvt", [D, V], BF16, kind="ExternalInput")
    wrt = nc.dram_tensor("wrt", [V, D], BF16, kind="ExternalInput")
    w1t = nc.dram_tensor("w1t", [D, M], BF16, kind="ExternalInput")
    w2t = nc.dram_tensor("w2t", [M, D], BF16, kind="ExternalInput")
    bq96 = nc.dram_tensor("bq96", [hk, H], F32, kind="ExternalInput")
    bv848 = nc.dram_tensor("bv848", [H, hv], F32, kind="ExternalInput")
    br6 = nc.dram_tensor("br6", [128, 6], F32, kind="ExternalInput")
    b1c = nc.dram_tensor("b1c", [128, 24], F32, kind="ExternalInput")
    b2v = nc.dram_tensor("b2v", [D], F32, kind="ExternalInput")
    ln1g = nc.dram_tensor("ln1g", [D], BF16, kind="ExternalInput")
    ln1b = nc.dram_tensor("ln1b", [D], BF16, kind="ExternalInput")
    ln2g = nc.dram_tensor("ln2g", [D], BF16, kind="ExternalInput")
    ln2b = nc.dram_tensor("ln2b", [D], BF16, kind="ExternalInput")
    mskd = nc.dram_tensor("msk", [hk, len(INCID), 128], BF16, kind="ExternalInput")
    identd = nc.dram_tensor("ident", [128, 128], BF16, kind="ExternalInput")
    out_d = nc.dram_tensor("out", [NB, L, D], F32, kind="ExternalOutput")

    def bcast(src, P, n):
        return bass.AP(tensor=src.tensor, offset=src.offset, ap=[[0, P], [1, n]])

    with tile.TileContext(nc) as tc:
        from contextlib import ExitStack
        with ExitStack() as top:
            wp = top.enter_context(tc.tile_pool(name="wts", bufs=1))
            dp = top.enter_context(tc.tile_pool(name="dram", bufs=2, space="DRAM"))

            # ---- resident weights (bf16), chunked [128, nchunks, cols]
            wk_sb = wp.tile([128, 6, K], BF16)
            nc.sync.dma_start(out=wk_sb, in_=wkt.rearrange("(c p) k -> p c k", p=128))
            wq_sb = wp.tile([128, 6, K], BF16)
            nc.sync.dma_start(out=wq_sb, in_=wqt.rearrange("(c p) k -> p c k", p=128))
            wv_sb = wp.tile([128, 6, V], BF16)
            nc.sync.dma_start(out=wv_sb, in_=wvt.rearrange("(c p) k -> p c k", p=128))
            wr_sb = wp.tile([128, 3, D], BF16)
            nc.sync.dma_start(out=wr_sb, in_=wrt.rearrange("(c p) k -> p c k", p=128))
            w1_sb = wp.tile([128, 6, M], BF16)
            nc.sync.dma_start(out=w1_sb, in_=w1t.rearrange("(c p) k -> p c k", p=128))
            w2_sb = wp.tile([128, 24, D], BF16)
            nc.sync.dma_start(out=w2_sb, in_=w2t.rearrange("(c p) k -> p c k", p=128))

            # ---- resident small constants
            bq_sb = wp.tile([hk, H], F32)
            nc.sync.dma_start(out=bq_sb, in_=bq96[:, :])
            bvb = wp.tile([hk, H, hv], F32)
            _bv = bv848[:, :]
            nc.sync.dma_start(out=bvb, in_=bass.AP(
                tensor=_bv.tensor, offset=_bv.offset, ap=[[0, hk], [hv, H], [1, hv]]))
            br_sb = wp.tile([128, 6], F32)
            nc.sync.dma_start(out=br_sb, in_=br6[:, :])
            b1_sb = wp.tile([128, 24], F32)
            nc.sync.dma_start(out=b1_sb, in_=b1c[:, :])
            b2b = wp.tile([128, D], F32)
            nc.sync.dma_start(out=b2b, in_=bcast(b2v[:], 128, D))
            g1b = wp.tile([128, D], BF16)
            nc.sync.dma_start(out=g1b, in_=bcast(ln1g[:], 128, D))
            b1lb = wp.tile([128, D], BF16)
            nc.sync.dma_start(out=b1lb, in_=bcast(ln1b[:], 128, D))
            g2b = wp.tile([128, D], BF16)
            nc.sync.dma_start(out=g2b, in_=bcast(ln2g[:], 128, D))
            b2lb = wp.tile([128, D], BF16)
            nc.sync.dma_start(out=b2lb, in_=bcast(ln2b[:], 128, D))
            msk_sb = wp.tile([hk, len(INCID), 128], BF16)
            nc.sync.dma_start(out=msk_sb, in_=mskd[:, :, :])
            ident = wp.tile([128, 128], BF16)
            nc.sync.dma_start(out=ident, in_=identd[:, :])
            eps_t = wp.tile([128, 1], F32)
            nc.vector.memset(eps_t, EPS)

            lnp = top.enter_context(tc.tile_pool(name="lnp", bufs=5))
            W = dict(
                wk=wk_sb, wq=wq_sb, wv=wv_sb, wr=wr_sb, w1=w1_sb, w2=w2_sb,
                bq=bq_sb, bvb=bvb, br=br_sb, b1=b1_sb, b2b=b2b,
                g1b=g1b, b1lb=b1lb, g2b=g2b, b2lb=b2lb,
                msk=msk_sb, ident=ident, eps=eps_t, lnp=lnp)
            scrs = []
            for e in range(NB):
                scrs.append({
                    "y": dp.tile([D * L], BF16, name="y_scr"),
                    "attn": dp.tile([D * L], BF16, name="attn_scr"),
                })
            # Pipelined emission: elem e+1's LN1 overlaps elem e's attention,
            # so the PE never waits on a standalone LN phase after startup.
            _emit_elem_ln1(nc, tc, 0, x_in[0], scrs[0], W)
            _emit_elem_attn(nc, tc, 0, x_in[0], out_d[0], scrs[0], W)
            _emit_elem_ln1(nc, tc, 1, x_in[1], scrs[1], W)
            _emit_elem_mlp(nc, tc, 0, x_in[0], out_d[0], scrs[0], W)
            _emit_elem_attn(nc, tc, 1, x_in[1], out_d[1], scrs[1], W)
            _emit_elem_mlp(nc, tc, 1, x_in[1], out_d[1], scrs[1], W)
    return nc


def _emit_elem_ln1(nc, tc, e, x_e, scr, W):
    """LN1: x -> y (bf16, [L, D] rows). Emitted early so elem e's LN overlaps
    the previous elem's attention/MLP passes (shared top-level pool)."""
    y_ld = scr["y"].rearrange("(l d) -> l d", d=D)
    lp = W["lnp"]
    for it in range((L + 127) // 128):
        l0 = it * 128
        p = min(128, L - l0)
        xt = lp.tile([128, D], F32, name="xt1")
        nc.sync.dma_start(out=xt[:p], in_=x_e[l0:l0 + p, :])
        y1 = lp.tile([128, D], BF16, name="y1")
        _layernorm_tile(nc, lp, xt, p, W["eps"], W["g1b"], W["b1lb"], y1)
        nc.sync.dma_start(out=y_ld[l0:l0 + p, :], in_=y1[:p])


def _emit_elem_attn(nc, tc, e, x_e, out_e, scr, W):
    from contextlib import ExitStack

    y_dl6 = scr["y"].rearrange("(c p l) -> p c l", p=128, l=L)
    attn_dl = scr["attn"].rearrange("(d l) -> d l", l=L)

    # small pool spanning passes A and B (ctxn/cpd)
    phAB = ExitStack()
    cp = phAB.enter_context(tc.tile_pool(name=f"pABc_{e}", bufs=1))

    # ============ pass A: K/V projection + linear-attn context ============
    with ExitStack() as phA:
        zp = phA.enter_context(tc.tile_pool(name=f"pAz_{e}", bufs=3))
        ep = phA.enter_context(tc.tile_pool(name=f"pAe_{e}", bufs=3))
        vp = phA.enter_context(tc.tile_pool(name=f"pAv_{e}", bufs=3))
        sp = phA.enter_context(tc.tile_pool(name=f"pAs_{e}", bufs=1))
        kp = phA.enter_context(tc.tile_pool(name=f"pAkp_{e}", bufs=2, space="PSUM"))
        vpp = phA.enter_context(tc.tile_pool(name=f"pAvp_{e}", bufs=2, space="PSUM"))
        cxp = phA.enter_context(tc.tile_pool(name=f"pAcx_{e}", bufs=1, space="PSUM"))

        ctx_ps = cxp.tile([hk, H, hv + 1], F32)
        ctx_flat = ctx_ps.rearrange("p a b -> p (a b)")
        zero96 = sp.tile([hk, hk], BF16)
        nc.vector.memset(zero96, 0.0)
        junk = sp.tile([hk, H * (hv + 1)], BF16)
        nc.vector.memset(junk, 0.0)
        # open the psum accumulation region with an all-zero write
        nc.tensor.matmul(out=ctx_flat, lhsT=zero96, rhs=junk, start=True, stop=False)

        for it5, l0, lw in _ltiles512():
            zsl = zp.tile([128, 6, 512], BF16, name="zsl")
            nc.sync.dma_start(out=zsl[:, :, :lw], in_=y_dl6[:, :, l0:l0 + lw])
            for sb in range((lw + 127) // 128):
                p = min(128, lw - sb * 128)
                lo = sb * 128
                kps = kp.tile([128, K], F32, name="kps")
                vps = vpp.tile([128, V], F32, name="vps")
                for c0, c1 in ((0, 512), (512, 768)):
                    for dc in range(6):
                        nc.tensor.matmul(out=kps[:p, c0:c1],
                                         lhsT=zsl[:, dc, lo:lo + p],
                                         rhs=W["wk"][:, dc, c0:c1],
                                         start=(dc == 0), stop=(dc == 5))
                for dc in range(6):
                    nc.tensor.matmul(out=vps[:p], lhsT=zsl[:, dc, lo:lo + p],
                                     rhs=W["wv"][:, dc, :],
                                     start=(dc == 0), stop=(dc == 5))
                ekt = ep.tile([128, K], BF16, name="ekt")
                nc.scalar.activation(out=ekt[:p], in_=kps[:p], func=Act.Exp)
                vt = vp.tile([128, H, hv + 1], BF16, name="vt")
                nc.vector.tensor_copy(
                    out=vt[:p, :, 0:hv],
                    in_=vps[:p].rearrange("p (a b) -> p a b", b=hv))
                nc.vector.memset(vt[:p, :, hv:hv + 1], 1.0)
                for h in range(H):
                    nc.tensor.matmul(out=ctx_ps[:, h, :],
                                     lhsT=ekt[:p, hk * h:hk * (h + 1)],
                                     rhs=vt[:p, h, :], start=False, stop=False)
        # close the accumulation region (+0)
        nc.tensor.matmul(out=ctx_flat, lhsT=zero96, rhs=junk, start=False, stop=True)

        # finalize: ctxn = ctx_raw / s + bv
        ctxs = sp.tile([hk, H, hv + 1], F32)
        nc.vector.tensor_copy(out=ctxs, in_=ctx_ps)
        ctxn = cp.tile([hk, H, hv], BF16)
        for h in range(H):
            rec = sp.tile([hk, 1], F32, name=f"rec{h}")
            nc.scalar.activation(out=rec, in_=ctxs[:, h, hv:hv + 1], func=Act.Ln)
            nc.scalar.activation(out=rec, in_=rec, func=Act.Exp, scale=-1.0)
            nc.vector.scalar_tensor_tensor(
                out=ctxn[:, h, :], in0=ctxs[:, h, 0:hv], scalar=rec,
                in1=W["bvb"][:, h, :], op0=Alu.mult, op1=Alu.add)

    # ============ pass B: Q proj + attention + reprojection ============
    with phAB as phB:
        cpd = cp.tile([hk, len(INCID), 128], BF16)
        nc.vector.memset(cpd, 0.0)
        for i, (c, h, jmin, jmax, dstp) in enumerate(INCID):
            nc.vector.tensor_copy(out=cpd[:, i, dstp:dstp + (jmax - jmin)],
                                  in_=ctxn[:, h, jmin:jmax])

        zp = phB.enter_context(tc.tile_pool(name=f"pBz_{e}", bufs=3))
        eqp = phB.enter_context(tc.tile_pool(name=f"pBe_{e}", bufs=2))
        rp = phB.enter_context(tc.tile_pool(name=f"pBr_{e}", bufs=2))
        ap_ = phB.enter_context(tc.tile_pool(name=f"pBa_{e}", bufs=2))
        rot = phB.enter_context(tc.tile_pool(name=f"pBo_{e}", bufs=2))
        qp = phB.enter_context(tc.tile_pool(name=f"pBqp_{e}", bufs=2, space="PSUM"))
        sqp = phB.enter_context(tc.tile_pool(name=f"pBsp_{e}", bufs=2, space="PSUM"))
        atp = phB.enter_context(tc.tile_pool(name=f"pBap_{e}", bufs=2, space="PSUM"))
        rop = phB.enter_context(tc.tile_pool(name=f"pBrp_{e}", bufs=2, space="PSUM"))

        for it5, l0, lw in _ltiles512():
            zt = zp.tile([128, 6, 512], BF16, name="zt")
            nc.sync.dma_start(out=zt[:, :, :lw], in_=y_dl6[:, :, l0:l0 + lw])
            eq = eqp.tile([hk, H, 512], BF16, name="eq")
            for h in range(H):
                qps = qp.tile([hk, 512], F32, name="qps")
                for dc in range(6):
                    nc.tensor.matmul(out=qps[:, :lw],
                                     lhsT=W["wq"][:, dc, hk * h:hk * (h + 1)],
                                     rhs=zt[:, dc, :lw],
                                     start=(dc == 0), stop=(dc == 5))
                nc.scalar.activation(out=eq[:, h, :lw], in_=qps[:, :lw],
                                     func=Act.Exp, bias=W["bq"][:, h:h + 1],
                                     scale=1.0)
            rqb = rp.tile([128, 3, 512], F32, name="rqb")
            attn_sb = ap_.tile([128, 3, 512], BF16, name="attn_sb")
            for c in range(3):
                inc = [i for i, t in enumerate(INCID) if t[0] == c]
                sqps = sqp.tile([128, 512], F32, name="sqps")
                for j, i in enumerate(inc):
                    h = INCID[i][1]
                    nc.tensor.matmul(out=sqps[:, :lw], lhsT=W["msk"][:, i, :],
                                     rhs=eq[:, h, :lw],
                                     start=(j == 0), stop=(j == len(inc) - 1))
                nc.scalar.activation(out=rqb[:, c, :lw], in_=sqps[:, :lw],
                                     func=Act.Ln)
                nc.scalar.activation(out=rqb[:, c, :lw], in_=rqb[:, c, :lw],
                                     func=Act.Exp, scale=-1.0)
                atps = atp.tile([128, 512], F32, name="atps")
                for j, i in enumerate(inc):
                    h = INCID[i][1]
                    nc.tensor.matmul(out=atps[:, :lw], lhsT=cpd[:, i, :],
                                     rhs=eq[:, h, :lw],
                                     start=(j == 0), stop=(j == len(inc) - 1))
                nc.vector.tensor_mul(out=attn_sb[:, c, :lw], in0=atps[:, :lw],
                                     in1=rqb[:, c, :lw])
            for dc in range(6):
                rops = rop.tile([128, 512], F32, name="rops")
                for c in range(3):
                    nc.tensor.matmul(out=rops[:, :lw],
                                     lhsT=W["wr"][:, c, dc * 128:(dc + 1) * 128],
                                     rhs=attn_sb[:, c, :lw],
                                     start=(c == 0), stop=(c == 2))
                ro = rot.tile([128, 512], BF16, name="ro")
                nc.vector.tensor_scalar_add(out=ro[:, :lw], in0=rops[:, :lw],
                                            scalar1=W["br"][:, dc:dc + 1])
                nc.sync.dma_start(out=attn_dl[dc * 128:(dc + 1) * 128, l0:l0 + lw],
                                  in_=ro[:, :lw])


def _emit_elem_mlp(nc, tc, e, x_e, out_e, scr, W):
    """Pass C: residual + LN2 + transpose + fc1/gelu + fc2 + residual."""
    from contextlib import ExitStack
    attn_ld = scr["attn"].rearrange("(l d) -> l d", d=D)
    with ExitStack() as phC:
        lp = phC.enter_context(tc.tile_pool(name=f"pCl_{e}", bufs=3))
        x2p = phC.enter_context(tc.tile_pool(name=f"pCx_{e}", bufs=2))
        y2p = phC.enter_context(tc.tile_pool(name=f"pCy_{e}", bufs=1))
        gp = phC.enter_context(tc.tile_pool(name=f"pCg_{e}", bufs=1))
        op = phC.enter_context(tc.tile_pool(name=f"pCo_{e}", bufs=2))
        tpp = phC.enter_context(tc.tile_pool(name=f"pCtp_{e}", bufs=1, space="PSUM"))
        f1p = phC.enter_context(tc.tile_pool(name=f"pCf1_{e}", bufs=3, space="PSUM"))
        f2p = phC.enter_context(tc.tile_pool(name=f"pCf2_{e}", bufs=2, space="PSUM"))

        for it5, l0, lw in _ltiles512():
            nsub = (lw + 127) // 128
            x2sl = x2p.tile([128, 4, D], F32, name="x2sl")
            y2sl = y2p.tile([128, 6, 512], BF16, name="y2sl")
            for sb in range(nsub):
                p = min(128, lw - sb * 128)
                lo = sb * 128
                gl0 = l0 + lo
                at = lp.tile([128, D], BF16, name="at")
                nc.sync.dma_start(out=at[:p], in_=attn_ld[gl0:gl0 + p, :])
                nc.sync.dma_start(out=x2sl[:p, sb, :], in_=x_e[gl0:gl0 + p, :])
                # x2 = attn + x  (f32 += bf16)
                nc.vector.tensor_add(out=x2sl[:p, sb, :], in0=x2sl[:p, sb, :],
                                     in1=at[:p])
                y2 = lp.tile([128, D], BF16, name="y2")
                _layernorm_tile(nc, lp, x2sl[:, sb, :], p, W["eps"],
                                W["g2b"], W["b2lb"], y2)
                tps = tpp.tile([128, 6, 128], BF16, name="tpsC")
                for dc in range(6):
                    nc.tensor.transpose(out=tps[:, dc, :p],
                                        in_=y2[:p, dc * 128:(dc + 1) * 128],
                                        identity=W["ident"][:p, :p])
                nc.vector.tensor_copy(out=y2sl[:, :, lo:lo + p], in_=tps[:, :, :p])
            G = gp.tile([128, 24, 512], BF16, name="G")
            for mc in range(24):
                f1 = f1p.tile([128, 512], F32, name="f1")
                for dc in range(6):
                    nc.tensor.matmul(out=f1[:, :lw],
                                     lhsT=W["w1"][:, dc, mc * 128:(mc + 1) * 128],
                                     rhs=y2sl[:, dc, :lw],
                                     start=(dc == 0), stop=(dc == 5))
                nc.scalar.activation(out=G[:, mc, :lw], in_=f1[:, :lw],
                                     func=Act.Gelu, bias=W["b1"][:, mc:mc + 1],
                                     scale=1.0)
            for sb in range(nsub):
                p = min(128, lw - sb * 128)
                lo = sb * 128
                gl0 = l0 + lo
                f2 = f2p.tile([128, D], F32, name="f2")
                for c0, c1 in ((0, 512), (512, 768)):
                    for mc in range(24):
                        nc.tensor.matmul(out=f2[:p, c0:c1],
                                         lhsT=G[:, mc, lo:lo + p],
                                         rhs=W["w2"][:, mc, c0:c1],
                                         start=(mc == 0), stop=(mc == 23))
                ot = op.tile([128, D], F32, name="ot")
                nc.vector.tensor_add(out=ot[:p], in0=f2[:p], in1=W["b2b"][:p])
                nc.vector.tensor_add(out=ot[:p], in0=ot[:p], in1=x2sl[:p, sb, :])
                nc.sync.dma_start(out=out_e[gl0:gl0 + p, :], in_=ot[:p])


def _legalize_single_wait(nc):
    """This walrus build encodes at most ONE sync wait per instruction
    (raw-bass style: waits are standalone InstEventSemaphore). Tile attaches
    multi-waits directly to instructions; hoist the extras onto EventSemaphore
    instructions inserted just before, on the same engine stream."""
    n = 0
    for f in nc.m.functions:
        for b in f.blocks:
            out = []
            changed = False
            for inst in b.instructions:
                si = inst.sync_info
                waits = list(si.on_wait) if si is not None and si.on_wait else []
                if len(waits) > 1:
                    changed = True
                    for w in waits[:-1]:
                        n += 1
                        ev = mybir.InstEventSemaphore(
                            name=f"EVLEG-{n}", ins=[], outs=[])
                        ev.engine = inst.engine
                        ev.sync_info = mybir.SyncInfo(on_wait=[w], on_update=[])
                        out.append(ev)
                    try:
                        si.on_wait = [waits[-1]]
                    except Exception:
                        inst.sync_info = mybir.SyncInfo(
                            on_wait=[waits[-1]],
                            on_update=list(si.on_update) if si.on_update else [])
                out.append(inst)
            if changed:
                b.instructions = out
    return n


_PROGRAM = None


def _get_program():
    global _PROGRAM
    if _PROGRAM is None:
        _PROGRAM = _build()
        _legalize_single_wait(_PROGRAM)
    return _PROGRAM


def _prep_common(inputs):
    f32 = np.float32
    g = lambda k: np.asarray(inputs[k], dtype=f32)
    msk = np.zeros((hk, len(INCID), 128), dtype=BF)
    for i, (c, h, jmin, jmax, dstp) in enumerate(INCID):
        msk[:, i, dstp:dstp + (jmax - jmin)] = 1
    return {
        "wkt": np.ascontiguousarray(g("Wk").T).astype(BF),
        "wqt": np.ascontiguousarray(g("Wq").T).astype(BF),
        "wvt": np.ascontiguousarray(g("Wv").T).astype(BF),
        "wrt": np.ascontiguousarray(g("Wr").T).astype(BF),
        "w1t": np.ascontiguousarray(g("W1").T).astype(BF),
        "w2t": np.ascontiguousarray(g("W2").T).astype(BF),
        "bq96": np.ascontiguousarray(g("bq").reshape(H, hk).T),
        "bv848": np.ascontiguousarray(g("bv").reshape(H, hv)),
        "br6": np.ascontiguousarray(g("br").reshape(6, 128).T),
        "b1c": np.ascontiguousarray(g("b1").reshape(24, 128).T),
        "b2v": g("b2"),
        "ln1g": g("ln1_g").astype(BF), "ln1b": g("ln1_b").astype(BF),
        "ln2g": g("ln2_g").astype(BF), "ln2b": g("ln2_b").astype(BF),
        "msk": msk,
        "ident": np.eye(128, dtype=BF),
    }


def kernel(**inputs):
    nc = _get_program()
    common = _prep_common(inputs)
    x = np.asarray(inputs["x"], dtype=np.float32)
    in_maps = [dict(common, x=np.ascontiguousarray(x[NB * i:NB * (i + 1)]))
               for i in range(NCORES)]
    res = run_bass_kernel_spmd(nc, in_maps, list(range(NCORES)))
    out = np.concatenate([res.results[i]["out"] for i in range(NCORES)], axis=0)
    return out.astype(np.float32)


if __name__ == "__main__":
    nc = _build()
    n = _legalize_single_wait(nc)
    print("built ok; hoisted waits:", n)
